# revision 32
# baseline (speedup 1.0000x reference)
"""Causal attention kernel for 8 TRN2 NeuronCores (Bass/Tile).

Problem: x [B=4, N=2048, Din=1024] f32, W_{q,k,v} [Dout=1024, Din] f32.
  q/k/v = x @ W.T ; S = q @ k.T (causal masked) ; P = softmax(S/sqrt(Dout)) ;
  out = P @ v.

Algebraic restructure (host precompute is free):
  S = (X Wq^T)(X Wk^T)^T = X (Wq^T Wk) X^T      -> W_qk = Wq^T Wk on host
  out = P (X Wv^T) = (P X) Wv^T
so the device never projects K or V. Per core:
  T1^T = W_qk^T X_q^T                 (fp8 DoubleRow, FD=512)
  S^T  = X^T(tiles) . T1^T            (bf16, causal chunks)
  P^T  = exp(S^T/sqrt(d)) * mask      (boundary tiles only)
  Z^T  = X^T . P^T                    (bf16, accumulated in PSUM)
  out  = (Z Wv^T) * 1/rowsum(P)       (bf16 epilogue)

Sharding: 8 cores = 4 batches x 2 halves; core half m owns 128-row query
blocks m, m+2, ..., m+14 (interleaved to balance causal work). One SPMD
program; per-core behavior comes only from data (xTq gather + masks).

T1 runs as fp8(e4m3) DoubleRow matmuls: wqk is host-scaled by 32 (values
land in e4m3 normal range) and pre-interleaved in dt-pairs; the extra 32x
is divided back out in the exp() scale. S/Z/VW matmuls stay bf16 (fp8
there would push rel-err past the gate; DoubleRow also doesn't pay at
FD=256 where LDWEIGHTS dominates).

All DMA rides the two hardware-DGE rings (sync + scalar): the gpsimd
software-DGE path measured ~20 GB/s effective and was both delaying T1's
weights and adding a ~12us output-store tail. Output stores are full
2KB-per-row [128, 1024] transfers.
"""

import math
from contextlib import ExitStack
from dataclasses import dataclass

import numpy as np
import ml_dtypes

import concourse.bass as bass
import concourse.mybir as mybir
import concourse.tile as tile
from concourse import bacc
from concourse.bass_utils import run_bass_kernel_spmd

P = 128
F32 = mybir.dt.float32
BF16 = mybir.dt.bfloat16
F8 = mybir.dt.float8e4
U8 = mybir.dt.uint8
NP_BF16 = ml_dtypes.bfloat16
NP_F8 = ml_dtypes.float8_e4m3

WSCALE = 32.0  # host pre-scale on wqk so fp8(e4m3) sees normal-range values
DR = mybir.MatmulPerfMode.DoubleRow


@dataclass(frozen=True)
class Cfg:
    SEQ: int = 2048   # kv sequence length per batch
    D: int = 1024     # Din == Dout
    R: int = 1024     # query rows handled per core
    CW: int = 256     # q-chunk width
    TS: int = 512     # T1 q-slab width (DoubleRow FD)

    @property
    def DT(self):  # contraction tiles
        return self.D // P

    @property
    def T(self):   # kv tiles
        return self.SEQ // P

    @property
    def NCH(self):  # query chunks per core
        return self.R // self.CW

    @property
    def NSL(self):  # T1 q-slabs per core
        return self.R // self.TS

    def ext(self, c):  # k-tile extent of chunk c (uniform across cores)
        return 4 * c + 4

    @property
    def n_mask_tiles(self):  # last 4 k-tiles of each chunk are masked
        return 4 * self.NCH

    @property
    def scale(self):
        return 1.0 / math.sqrt(self.D)


# q-block (128-row) assignment per core half m
def q_blocks(cfg: Cfg, m: int):
    nb_total = cfg.SEQ // P
    return list(range(m, nb_total, 2))


def _emit(ctx: ExitStack, tc: tile.TileContext, cfg: Cfg, aps):
    nc = tc.nc
    DT, T, CW, NCH, D, SEQ, TS = (
        cfg.DT, cfg.T, cfg.CW, cfg.NCH, cfg.D, cfg.SEQ, cfg.TS)
    DTP = DT // 2  # DoubleRow dt-pairs

    xT, x_n, xTq8, wqk8, wvT, mask, o_ap = (
        aps["xT"], aps["x"], aps["xTq8"], aps["wqk8"], aps["wvT"],
        aps["mask"], aps["o"],
    )

    # ---- SBUF pools ----
    cpool = ctx.enter_context(tc.tile_pool(name="consts", bufs=1))
    wqk_p = ctx.enter_context(tc.tile_pool(name="wqk", bufs=1))
    xTq_p = ctx.enter_context(tc.tile_pool(name="xTq", bufs=1))
    t1_p = ctx.enter_context(tc.tile_pool(name="t1", bufs=1))
    xT_p = ctx.enter_context(tc.tile_pool(name="xTs", bufs=1))
    x_p = ctx.enter_context(tc.tile_pool(name="xs", bufs=1))
    wv_p = ctx.enter_context(tc.tile_pool(name="wv", bufs=1))
    ppool = ctx.enter_context(tc.tile_pool(name="pT", bufs=24))
    zt_p = ctx.enter_context(tc.tile_pool(name="zt", bufs=3))
    mpool = ctx.enter_context(tc.tile_pool(name="mt", bufs=1))
    spool = ctx.enter_context(tc.tile_pool(name="stage", bufs=2))
    rpool = ctx.enter_context(tc.tile_pool(name="rcp", bufs=4))
    apool = ctx.enter_context(tc.tile_pool(name="acc", bufs=4))
    # ---- PSUM pools ----
    psS = ctx.enter_context(tc.tile_pool(name="psS", bufs=2, space="PSUM"))
    psZ = ctx.enter_context(tc.tile_pool(name="psZ", bufs=2, space="PSUM"))
    psO = ctx.enter_context(tc.tile_pool(name="psO", bufs=2, space="PSUM"))

    ones_b = cpool.tile([P, 1], BF16, tag="ones_b")
    nc.vector.memset(ones_b, 1.0)

    # warm the PE p-state ramp on dummy data while the first loads land
    warm = cpool.tile([P, P], BF16, tag="warm")
    nc.vector.memset(warm, 0.0)
    psw = psS.tile([P, CW], F32, tag="psS", name="warm")
    for i in range(30):
        nc.tensor.matmul(psw[:, 0:P], warm, warm, start=True, stop=True)

    mask_sb = mpool.tile([P, cfg.n_mask_tiles, CW], F8, tag="mt")

    # ---- resident loads (all on the two hardware-DGE rings) ----
    wqk8_sb = wqk_p.tile([P, DT, DTP, 2, P], F8, tag="wqk8")
    xTq8_sb = xTq_p.tile([P, cfg.NSL, DTP, 2, TS], F8, tag="xTq8")
    t1_sb = t1_p.tile([P, DT, cfg.R], BF16, tag="t1")
    xT_sb = xT_p.tile([P, DT, SEQ], BF16, tag="xTs")
    x_sb = x_p.tile([P, T, D], BF16, tag="xs")
    wv_sb = wv_p.tile([P, DT, D], BF16, tag="wv")

    rr_mask = mask.rearrange("p (n w) -> p n w", n=cfg.n_mask_tiles)
    rr_xT = xT.rearrange("(dt p) k -> p dt k", p=P)
    rr_x = x_n.rearrange("(t p) d -> p t d", p=P)
    rr_wqk8 = wqk8.rearrange("p (o dtp i w) -> p o dtp i w", o=DT, dtp=DTP, i=2)
    rr_xTq8 = [
        xTq8[s].rearrange("p (dtp i w) -> p dtp i w", dtp=DTP, i=2)
        for s in range(cfg.NSL)
    ]

    # The front-end is HBM-limited: ~14MB of residents must land in the
    # first ~45us, right at the per-core roofline. Order each ring by
    # consumption deadline. The scalar ring carries only early-criticals
    # (its engine also runs the exps — a late blocking DIRECT2D issue
    # would stall softmax); everything else rides sync.
    def wqk_load(g):
        nc.scalar.dma_start(
            wqk8_sb[:, 2 * g:2 * g + 2], rr_wqk8[:, 2 * g:2 * g + 2])

    # wqk o-pair 0 leads the scalar ring so T1's first LDWEIGHTS is fed
    # by ~10us
    wqk_load(0)
    wqk_load(1)
    wqk_load(2)
    wqk_load(3)
    nc.scalar.dma_start(mask_sb[:, 0:8, :], rr_mask[:, 0:8, :])
    nc.scalar.dma_start(mask_sb[:, 8:, :], rr_mask[:, 8:, :])
    # sync ring carries the big stream solo (two heavy rings split the
    # HBM share and delayed the critical prefix), strict deadline order.
    # xTq8 slab 0 split in two so T1's first matmul only waits ~250KB;
    # wv is late-deadline (VW trails by two chunks).
    for dtp in range(4):
        nc.sync.dma_start(
            xTq8_sb[:, 0, dtp:dtp + 1], rr_xTq8[0][:, dtp:dtp + 1])
    nc.sync.dma_start(xT_sb[:, :, 0:P], rr_xT[:, :, 0:P])
    nc.sync.dma_start(xT_sb[:, :, P:4 * P], rr_xT[:, :, P:4 * P])
    nc.sync.dma_start(x_sb[:, 0:1, :], rr_x[:, 0:1, :])
    nc.sync.dma_start(x_sb[:, 1:4, :], rr_x[:, 1:4, :])
    nc.sync.dma_start(xT_sb[:, :, 4 * P:8 * P], rr_xT[:, :, 4 * P:8 * P])
    nc.sync.dma_start(x_sb[:, 4:8, :], rr_x[:, 4:8, :])
    nc.sync.dma_start(xTq8_sb[:, 1], rr_xTq8[1])
    nc.sync.dma_start(xT_sb[:, :, 8 * P:12 * P], rr_xT[:, :, 8 * P:12 * P])
    nc.sync.dma_start(x_sb[:, 8:12, :], rr_x[:, 8:12, :])
    nc.sync.dma_start(wv_sb, wvT.rearrange("(dt p) o -> p dt o", p=P))
    nc.sync.dma_start(xT_sb[:, :, 12 * P:], rr_xT[:, :, 12 * P:])
    nc.sync.dma_start(x_sb[:, 12:, :], rr_x[:, 12:, :])

    # ---- T1^T = W_qk^T X_q^T per 512-q slab: fp8 DoubleRow, FD=512 ----
    def do_t1(s):
        for o in range(DT):
            ps = psO.tile([P, TS], F32, tag="psO", name=f"t1_{s}_{o}")
            for dtp in range(DTP):
                nc.tensor.matmul(
                    ps, wqk8_sb[:, o, dtp], xTq8_sb[:, s, dtp],
                    start=(dtp == 0), stop=(dtp == DTP - 1), perf_mode=DR)
            nc.vector.tensor_copy(t1_sb[:, o, s * TS:(s + 1) * TS], ps)

    # ---- attention per q-chunk; VW epilogue trails by 1 chunk so wv and
    # the zt/rcp pipelines never stall the PE ----
    pend = {}

    def do_chunk(c):
        E = cfg.ext(c)
        qs = slice(c * CW, (c + 1) * CW)

        # S^T tiles -> pT (exp + boundary masks); acc = sum_t pT for the
        # softmax denominators (masks zero beyond-diagonal contributions).
        # In the chunk's last two kv-tiles the FIRST q-half (the lower
        # query block) lies entirely above the diagonal on both core
        # halves, so S/Z run them at N=128 on the second q-half only.
        pTs = []
        acc = apool.tile([P, CW], F32, tag="acc", name=f"acc{c}")
        for t in range(E):
            half = t >= E - 2
            w = P if half else CW
            q0 = c * CW + (P if half else 0)
            ps = psS.tile([P, CW], F32, tag="psS", name=f"s_{c}_{t}")
            for dt in range(DT):
                nc.tensor.matmul(
                    ps[:, 0:w], xT_sb[:, dt, t * P:(t + 1) * P],
                    t1_sb[:, dt, q0:q0 + w],
                    start=(dt == 0), stop=(dt == DT - 1))
            pT = ppool.tile([P, CW], BF16, tag="pT", name=f"pT_{c}_{t}")
            nc.scalar.activation(
                pT[:, 0:w], ps[:, 0:w], mybir.ActivationFunctionType.Exp,
                scale=cfg.scale / WSCALE)
            if t >= E - 4:
                nc.vector.tensor_mul(
                    pT[:, 0:w], pT[:, 0:w],
                    mask_sb[:, 4 * c + (t - (E - 4)), 0:w])
            if t == 0:
                nc.vector.tensor_copy(acc, pT)
            elif half:
                nc.vector.tensor_add(acc[:, P:], acc[:, P:], pT[:, 0:P])
            else:
                nc.vector.tensor_add(acc, acc, pT)
            pTs.append((pT, half))

        # Z^T = X^T P^T accumulated in PSUM, two d-half passes
        zt_sb = zt_p.tile([P, DT, CW], BF16, tag="zt", name=f"zt{c}")
        H = DT // 2
        for h in range(2):
            pz = psZ.tile([P, H, CW], F32, tag="psZ", name=f"pz{c}_{h}")
            # o-outer: accumulation groups sharing a PSUM bank must be
            # strictly sequential (one open group per 2KB zero region)
            for o in range(H):
                d0 = (h * H + o) * P
                for t in range(E):
                    pT, half = pTs[t]
                    if half:
                        nc.tensor.matmul(
                            pz[:, o, P:], x_sb[:, t, d0:d0 + P], pT[:, 0:P],
                            start=False, stop=(t == E - 1),
                            skip_group_check=True)
                    else:
                        nc.tensor.matmul(
                            pz[:, o, :], x_sb[:, t, d0:d0 + P], pT,
                            start=(t == 0), stop=False,
                            skip_group_check=True)
            nc.vector.tensor_copy(zt_sb[:, h * H:(h + 1) * H, :], pz)

        # denominators: one ones-matmul per block over the accumulated pT
        accb = apool.tile([P, CW], BF16, tag="accb", name=f"accb{c}")
        nc.vector.tensor_copy(accb, acc)
        psd = psS.tile([P, CW], F32, tag="psS", name=f"psd{c}")
        rcp = rpool.tile([P, 2], F32, tag="rcp", name=f"rcp{c}")
        for j in range(2):
            nc.tensor.matmul(
                psd[:, j:j + 1], accb[:, j * P:(j + 1) * P], ones_b,
                start=True, stop=True)
            nc.vector.reciprocal(rcp[:, j:j + 1], psd[:, j:j + 1])
        pend[c] = (zt_sb, rcp)

    def do_vw(c):
        zt_sb, rcp = pend.pop(c)
        for j in range(2):
            osb = spool.tile([P, D], BF16, tag="osb", name=f"osb{c}_{j}")
            r0 = c * CW + j * P
            for h in range(2):
                po = psO.tile([P, D // 2], F32, tag="psO")
                for dt in range(DT):
                    nc.tensor.matmul(
                        po, zt_sb[:, dt, j * P:(j + 1) * P],
                        wv_sb[:, dt, h * (D // 2):(h + 1) * (D // 2)],
                        start=(dt == 0), stop=(dt == DT - 1))
                hs = slice(h * (D // 2), (h + 1) * (D // 2))
                # h-halves scaled out on different engines so the copies
                # overlap (scalar alone serialized the store tail); h=1,
                # the last copy before the store, is itself split so
                # vector and scalar each do half
                if h == 0:
                    nc.vector.tensor_scalar_mul(
                        osb[:, hs], po, rcp[:, j:j + 1])
                else:
                    q4 = D // 4
                    nc.vector.tensor_scalar_mul(
                        osb[:, 2 * q4:3 * q4], po[:, 0:q4], rcp[:, j:j + 1])
                    nc.scalar.activation(
                        osb[:, 3 * q4:], po[:, q4:],
                        mybir.ActivationFunctionType.Copy,
                        scale=rcp[:, j:j + 1])

            # full-row 2KB-per-partition stores on the sync hardware-DGE
            # ring (loads there are done by ~50us; keeping stores off the
            # scalar engine protects the exp/copy pipeline from blocking
            # DIRECT2D issues). The last chunk's stores are on the
            # critical tail: j=1 ships per-h above; j=0 in two row-groups
            # so two queues drain it in parallel.
            # the very last store goes via scalar (its copies ran there,
            # no exps remain to protect); everything else rides sync
            eng = nc.scalar if (c == NCH - 1 and j == 1) else nc.sync
            eng.dma_start(o_ap[r0:r0 + P, :], osb)

    # T1 slab 1 is deferred until after chunk 1 (its consumers are chunks
    # 2-3); VW trails by TWO chunks so wv's 2MB load leaves the HBM-
    # saturated front window
    do_t1(0)
    do_chunk(0)
    do_chunk(1)
    do_t1(1)
    do_chunk(2)
    do_vw(0)
    do_chunk(3)
    do_vw(1)
    do_vw(2)
    do_vw(3)


def build_program(cfg: Cfg):
    nc = bacc.Bacc("TRN2", dynamic_dma_scratch_size=2048)
    aps = {
        "xT": nc.dram_tensor("xT", [cfg.D, cfg.SEQ], BF16, kind="ExternalInput").ap(),
        "x": nc.dram_tensor("x", [cfg.SEQ, cfg.D], BF16, kind="ExternalInput").ap(),
        "xTq8": nc.dram_tensor(
            "xTq8", [cfg.NSL, P, cfg.DT * cfg.TS], F8,
            kind="ExternalInput").ap(),
        "wqk8": nc.dram_tensor(
            "wqk8", [P, cfg.DT * cfg.DT * P], F8, kind="ExternalInput").ap(),
        "wvT": nc.dram_tensor("wvT", [cfg.D, cfg.D], BF16, kind="ExternalInput").ap(),
        "mask": nc.dram_tensor(
            "mask", [P, cfg.n_mask_tiles * cfg.CW], F8,
            kind="ExternalInput").ap(),
        "o": nc.dram_tensor("o", [cfg.R, cfg.D], BF16, kind="ExternalOutput").ap(),
    }
    with tile.TileContext(nc) as tc:
        with ExitStack() as ctx:
            _emit(ctx, tc, cfg, aps)
    nc.compile()
    return nc


def make_mask(cfg: Cfg, qglob: np.ndarray) -> np.ndarray:
    """fp8 mask tiles for the last 4 k-tiles of each chunk: 1 = keep.
    Layout [P, n_mask_tiles*CW] (partition-major for one wide DMA)."""
    m = np.zeros((cfg.n_mask_tiles, P, cfg.CW), dtype=NP_F8)
    for c in range(cfg.NCH):
        qg = qglob[c * cfg.CW:(c + 1) * cfg.CW]  # [CW]
        E = cfg.ext(c)
        for i, t in enumerate(range(E - 4, E)):
            kg = np.arange(t * P, (t + 1) * P)  # [P]
            if i >= 2:
                # kernel computes only the second q-half for these tiles;
                # its mask sits in the slot's first 128 columns
                m[4 * c + i][:, 0:P] = (
                    kg[:, None] <= qg[None, P:]).astype(NP_F8)
            else:
                m[4 * c + i] = (kg[:, None] <= qg[None, :]).astype(NP_F8)
    return np.ascontiguousarray(m.transpose(1, 0, 2).reshape(P, -1))


def pack_dr_pairs(a: np.ndarray, w: int) -> np.ndarray:
    """[D, C] f32 -> fp8 [C//w, P, DT*w] with dt-pair interleave for
    DoubleRow: out[s, p, (dtp*2+i)*w + j] = a[(2*dtp+i)*P + p, s*w + j]."""
    Dd, C = a.shape
    DTP = Dd // P // 2
    return np.ascontiguousarray(
        a.reshape(DTP, 2, P, C // w, w)
        .transpose(3, 2, 0, 1, 4)
        .reshape(C // w, P, -1)).astype(NP_F8)


def make_core_inputs(cfg: Cfg, xT_bf, x_bf, x_f32, wqk8, wvT_bf, m: int):
    blocks = q_blocks(cfg, m)
    qglob = np.concatenate([np.arange(b * P, (b + 1) * P) for b in blocks])
    # xTq8[s, p, (dtp*2+i)*TS + j] = x[qglob[s*TS+j], (2*dtp+i)*P + p]
    xTq8 = pack_dr_pairs(np.ascontiguousarray(x_f32[qglob].T), cfg.TS)
    return {
        "xT": xT_bf,
        "x": x_bf,
        "xTq8": xTq8,
        "wqk8": wqk8,
        "wvT": wvT_bf,
        "mask": make_mask(cfg, qglob),
    }, qglob


_prog_cache = {}


def get_program(cfg: Cfg):
    if cfg not in _prog_cache:
        _prog_cache[cfg] = build_program(cfg)
    return _prog_cache[cfg]


def run(x, W_query, W_key, W_value, trace=False, trace_cores=None):
    """Returns (out [B, N, D], BassKernelResults)."""
    cfg = Cfg()
    B = x.shape[0]
    nc = get_program(cfg)
    x = np.asarray(x, dtype=np.float32)
    Wq = np.asarray(W_query, dtype=np.float32)
    Wk = np.asarray(W_key, dtype=np.float32)
    Wv = np.asarray(W_value, dtype=np.float32)
    # wqk8[p, ((o*DTP+dtp)*2+i)*P + col] = 32*wqk[(2dtp+i)*P+p, o*P+col]
    W32 = (WSCALE * (Wq.T @ Wk)).astype(np.float32)
    wqk8 = np.ascontiguousarray(
        W32.reshape(cfg.DT // 2, 2, P, cfg.DT, P)
        .transpose(2, 3, 0, 1, 4)
        .reshape(P, -1)).astype(NP_F8)
    wvT_bf = np.ascontiguousarray(Wv.T).astype(NP_BF16)

    in_maps = []
    qglobs = []
    for core in range(2 * B):
        b, m = core // 2, core % 2
        if m == 0:
            x_bf = x[b].astype(NP_BF16)
            xT_bf = np.ascontiguousarray(x[b].T).astype(NP_BF16)
        im, qglob = make_core_inputs(cfg, xT_bf, x_bf, x[b], wqk8, wvT_bf, m)
        in_maps.append(im)
        qglobs.append(qglob)

    res = run_bass_kernel_spmd(
        nc, in_maps, list(range(2 * B)), trace=trace,
        trace_cores=trace_cores)

    out = np.empty((B, cfg.SEQ, cfg.D), dtype=np.float32)
    for core in range(2 * B):
        b = core // 2
        out[b][qglobs[core]] = res.results[core]["o"].astype(np.float32)
    return out, res


def kernel(**inputs) -> np.ndarray:
    out, _ = run(
        inputs["x"], inputs["W_query"], inputs["W_key"], inputs["W_value"])
    return out


# revision 33
# speedup vs baseline: 1.0154x; 1.0154x over previous
"""Causal attention kernel for 8 TRN2 NeuronCores (Bass/Tile).

Problem: x [B=4, N=2048, Din=1024] f32, W_{q,k,v} [Dout=1024, Din] f32.
  q/k/v = x @ W.T ; S = q @ k.T (causal masked) ; P = softmax(S/sqrt(Dout)) ;
  out = P @ v.

Algebraic restructure (host precompute is free):
  S = (X Wq^T)(X Wk^T)^T = X (Wq^T Wk) X^T      -> W_qk = Wq^T Wk on host
  out = P (X Wv^T) = (P X) Wv^T
so the device never projects K or V. Per core:
  T1^T = W_qk^T X_q^T                 (fp8 DoubleRow, FD=512)
  S^T  = X^T(tiles) . T1^T            (bf16, causal chunks)
  P^T  = exp(S^T/sqrt(d)) * mask      (boundary tiles only)
  Z^T  = X^T . P^T                    (bf16, accumulated in PSUM)
  out  = (Z Wv^T) * 1/rowsum(P)       (bf16 epilogue)

Sharding: 8 cores = 4 batches x 2 halves; core half m owns 128-row query
blocks m, m+2, ..., m+14 (interleaved to balance causal work). One SPMD
program; per-core behavior comes only from data (xTq gather + masks).

T1 runs as fp8(e4m3) DoubleRow matmuls: wqk is host-scaled by 32 (values
land in e4m3 normal range) and pre-interleaved in dt-pairs; the extra 32x
is divided back out in the exp() scale. S/Z/VW matmuls stay bf16 (fp8
there would push rel-err past the gate; DoubleRow also doesn't pay at
FD=256 where LDWEIGHTS dominates).

All DMA rides the two hardware-DGE rings (sync + scalar): the gpsimd
software-DGE path measured ~20 GB/s effective and was both delaying T1's
weights and adding a ~12us output-store tail. Output stores are full
2KB-per-row [128, 1024] transfers.
"""

import math
from contextlib import ExitStack
from dataclasses import dataclass

import numpy as np
import ml_dtypes

import concourse.bass as bass
import concourse.mybir as mybir
import concourse.tile as tile
from concourse import bacc
from concourse.bass_utils import run_bass_kernel_spmd

P = 128
F32 = mybir.dt.float32
BF16 = mybir.dt.bfloat16
F8 = mybir.dt.float8e4
U8 = mybir.dt.uint8
NP_BF16 = ml_dtypes.bfloat16
NP_F8 = ml_dtypes.float8_e4m3

WSCALE = 32.0  # host pre-scale on wqk so fp8(e4m3) sees normal-range values
DR = mybir.MatmulPerfMode.DoubleRow


@dataclass(frozen=True)
class Cfg:
    SEQ: int = 2048   # kv sequence length per batch
    D: int = 1024     # Din == Dout
    R: int = 1024     # query rows handled per core
    CW: int = 256     # q-chunk width
    TS: int = 512     # T1 q-slab width (DoubleRow FD)

    @property
    def DT(self):  # contraction tiles
        return self.D // P

    @property
    def T(self):   # kv tiles
        return self.SEQ // P

    @property
    def NCH(self):  # query chunks per core
        return self.R // self.CW

    @property
    def NSL(self):  # T1 q-slabs per core
        return self.R // self.TS

    def ext(self, c):  # k-tile extent of chunk c (uniform across cores)
        return 4 * c + 4

    @property
    def n_mask_tiles(self):  # last 4 k-tiles of each chunk are masked
        return 4 * self.NCH

    @property
    def scale(self):
        return 1.0 / math.sqrt(self.D)


# q-block (128-row) assignment per core half m
def q_blocks(cfg: Cfg, m: int):
    nb_total = cfg.SEQ // P
    return list(range(m, nb_total, 2))


def _emit(ctx: ExitStack, tc: tile.TileContext, cfg: Cfg, aps):
    nc = tc.nc
    DT, T, CW, NCH, D, SEQ, TS = (
        cfg.DT, cfg.T, cfg.CW, cfg.NCH, cfg.D, cfg.SEQ, cfg.TS)
    DTP = DT // 2  # DoubleRow dt-pairs

    xT, x_n, xTq8, wqk8, wvT, mask, o_ap = (
        aps["xT"], aps["x"], aps["xTq8"], aps["wqk8"], aps["wvT"],
        aps["mask"], aps["o"],
    )

    # ---- SBUF pools ----
    cpool = ctx.enter_context(tc.tile_pool(name="consts", bufs=1))
    wqk_p = ctx.enter_context(tc.tile_pool(name="wqk", bufs=1))
    xTq_p = ctx.enter_context(tc.tile_pool(name="xTq", bufs=1))
    t1_p = ctx.enter_context(tc.tile_pool(name="t1", bufs=1))
    xT_p = ctx.enter_context(tc.tile_pool(name="xTs", bufs=1))
    x_p = ctx.enter_context(tc.tile_pool(name="xs", bufs=1))
    wv_p = ctx.enter_context(tc.tile_pool(name="wv", bufs=1))
    ppool = ctx.enter_context(tc.tile_pool(name="pT", bufs=24))
    zt_p = ctx.enter_context(tc.tile_pool(name="zt", bufs=3))
    mpool = ctx.enter_context(tc.tile_pool(name="mt", bufs=1))
    spool = ctx.enter_context(tc.tile_pool(name="stage", bufs=2))
    rpool = ctx.enter_context(tc.tile_pool(name="rcp", bufs=4))
    apool = ctx.enter_context(tc.tile_pool(name="acc", bufs=4))
    # ---- PSUM pools ----
    psS = ctx.enter_context(tc.tile_pool(name="psS", bufs=2, space="PSUM"))
    psZ = ctx.enter_context(tc.tile_pool(name="psZ", bufs=2, space="PSUM"))
    psO = ctx.enter_context(tc.tile_pool(name="psO", bufs=2, space="PSUM"))

    ones_b = cpool.tile([P, 1], BF16, tag="ones_b")
    nc.vector.memset(ones_b, 1.0)

    # warm the PE p-state ramp on dummy data while the first loads land
    warm = cpool.tile([P, P], BF16, tag="warm")
    nc.vector.memset(warm, 0.0)
    psw = psS.tile([P, CW], F32, tag="psS", name="warm")
    for i in range(30):
        nc.tensor.matmul(psw[:, 0:P], warm, warm, start=True, stop=True)

    mask_sb = mpool.tile([P, cfg.n_mask_tiles, CW], F8, tag="mt")

    # ---- resident loads (all on the two hardware-DGE rings) ----
    wqk8_sb = wqk_p.tile([P, DT, DTP, 2, P], F8, tag="wqk8")
    xTq8_sb = xTq_p.tile([P, cfg.NSL, DTP, 2, TS], F8, tag="xTq8")
    t1_sb = t1_p.tile([P, DT, cfg.R], BF16, tag="t1")
    xT_sb = xT_p.tile([P, DT, SEQ], BF16, tag="xTs")
    x_sb = x_p.tile([P, T, D], BF16, tag="xs")
    wv_sb = wv_p.tile([P, DT, D], BF16, tag="wv")

    rr_mask = mask.rearrange("p (n w) -> p n w", n=cfg.n_mask_tiles)
    rr_xT = xT.rearrange("(dt p) k -> p dt k", p=P)
    rr_x = x_n.rearrange("(t p) d -> p t d", p=P)
    rr_wqk8 = wqk8.rearrange("p (o dtp i w) -> p o dtp i w", o=DT, dtp=DTP, i=2)
    rr_xTq8 = [
        xTq8[s].rearrange("p (dtp i w) -> p dtp i w", dtp=DTP, i=2)
        for s in range(cfg.NSL)
    ]

    # The front-end is HBM-limited: ~14MB of residents must land in the
    # first ~45us, right at the per-core roofline. Order each ring by
    # consumption deadline. The scalar ring carries only early-criticals
    # (its engine also runs the exps — a late blocking DIRECT2D issue
    # would stall softmax); everything else rides sync.
    def wqk_load(g):
        nc.scalar.dma_start(
            wqk8_sb[:, 2 * g:2 * g + 2], rr_wqk8[:, 2 * g:2 * g + 2])

    # wqk o-pair 0 leads the scalar ring so T1's first LDWEIGHTS is fed
    # by ~10us
    wqk_load(0)
    wqk_load(1)
    wqk_load(2)
    wqk_load(3)
    nc.scalar.dma_start(mask_sb[:, 0:8, :], rr_mask[:, 0:8, :])
    nc.scalar.dma_start(mask_sb[:, 8:, :], rr_mask[:, 8:, :])
    # sync ring carries the big stream solo (two heavy rings split the
    # HBM share and delayed the critical prefix), strict deadline order.
    # xTq8 slab 0 split in two so T1's first matmul only waits ~250KB;
    # wv is late-deadline (VW trails by two chunks).
    for dtp in range(4):
        nc.sync.dma_start(
            xTq8_sb[:, 0, dtp:dtp + 1], rr_xTq8[0][:, dtp:dtp + 1])
    nc.sync.dma_start(xT_sb[:, :, 0:P], rr_xT[:, :, 0:P])
    nc.sync.dma_start(xT_sb[:, :, P:4 * P], rr_xT[:, :, P:4 * P])
    nc.sync.dma_start(x_sb[:, 0:1, :], rr_x[:, 0:1, :])
    nc.sync.dma_start(x_sb[:, 1:4, :], rr_x[:, 1:4, :])
    nc.sync.dma_start(xT_sb[:, :, 4 * P:8 * P], rr_xT[:, :, 4 * P:8 * P])
    nc.sync.dma_start(x_sb[:, 4:8, :], rr_x[:, 4:8, :])
    nc.sync.dma_start(xTq8_sb[:, 1], rr_xTq8[1])
    nc.sync.dma_start(xT_sb[:, :, 8 * P:12 * P], rr_xT[:, :, 8 * P:12 * P])
    nc.sync.dma_start(x_sb[:, 8:12, :], rr_x[:, 8:12, :])
    nc.sync.dma_start(wv_sb, wvT.rearrange("(dt p) o -> p dt o", p=P))
    nc.sync.dma_start(xT_sb[:, :, 12 * P:], rr_xT[:, :, 12 * P:])
    nc.sync.dma_start(x_sb[:, 12:, :], rr_x[:, 12:, :])

    # ---- T1^T = W_qk^T X_q^T per 512-q slab: fp8 DoubleRow, FD=512 ----
    def do_t1(s):
        for o in range(DT):
            ps = psO.tile([P, TS], F32, tag="psO", name=f"t1_{s}_{o}")
            for dtp in range(DTP):
                nc.tensor.matmul(
                    ps, wqk8_sb[:, o, dtp], xTq8_sb[:, s, dtp],
                    start=(dtp == 0), stop=(dtp == DTP - 1), perf_mode=DR)
            nc.vector.tensor_copy(t1_sb[:, o, s * TS:(s + 1) * TS], ps)

    # ---- attention per q-chunk; VW epilogue trails by 1 chunk so wv and
    # the zt/rcp pipelines never stall the PE ----
    pend = {}

    def do_chunk(c):
        E = cfg.ext(c)
        qs = slice(c * CW, (c + 1) * CW)

        # S^T tiles -> pT (exp + boundary masks); acc = sum_t pT for the
        # softmax denominators (masks zero beyond-diagonal contributions).
        # In the chunk's last two kv-tiles the FIRST q-half (the lower
        # query block) lies entirely above the diagonal on both core
        # halves, so S/Z run them at N=128 on the second q-half only.
        pTs = []
        acc = apool.tile([P, CW], F32, tag="acc", name=f"acc{c}")
        for t in range(E):
            half = t >= E - 2
            w = P if half else CW
            q0 = c * CW + (P if half else 0)
            ps = psS.tile([P, CW], F32, tag="psS", name=f"s_{c}_{t}")
            for dt in range(DT):
                nc.tensor.matmul(
                    ps[:, 0:w], xT_sb[:, dt, t * P:(t + 1) * P],
                    t1_sb[:, dt, q0:q0 + w],
                    start=(dt == 0), stop=(dt == DT - 1))
            pT = ppool.tile([P, CW], BF16, tag="pT", name=f"pT_{c}_{t}")
            nc.scalar.activation(
                pT[:, 0:w], ps[:, 0:w], mybir.ActivationFunctionType.Exp,
                scale=cfg.scale / WSCALE)
            if t >= E - 4:
                nc.vector.tensor_mul(
                    pT[:, 0:w], pT[:, 0:w],
                    mask_sb[:, 4 * c + (t - (E - 4)), 0:w])
            if t == 0:
                nc.vector.tensor_copy(acc, pT)
            elif half:
                nc.vector.tensor_add(acc[:, P:], acc[:, P:], pT[:, 0:P])
            else:
                nc.vector.tensor_add(acc, acc, pT)
            pTs.append((pT, half))

        # Z^T = X^T P^T accumulated in PSUM, two d-half passes
        zt_sb = zt_p.tile([P, DT, CW], BF16, tag="zt", name=f"zt{c}")
        H = DT // 2
        for h in range(2):
            pz = psZ.tile([P, H, CW], F32, tag="psZ", name=f"pz{c}_{h}")
            # o-outer: accumulation groups sharing a PSUM bank must be
            # strictly sequential (one open group per 2KB zero region)
            for o in range(H):
                d0 = (h * H + o) * P
                for t in range(E):
                    pT, half = pTs[t]
                    if half:
                        nc.tensor.matmul(
                            pz[:, o, P:], x_sb[:, t, d0:d0 + P], pT[:, 0:P],
                            start=False, stop=(t == E - 1),
                            skip_group_check=True)
                    else:
                        nc.tensor.matmul(
                            pz[:, o, :], x_sb[:, t, d0:d0 + P], pT,
                            start=(t == 0), stop=False,
                            skip_group_check=True)
            nc.vector.tensor_copy(zt_sb[:, h * H:(h + 1) * H, :], pz)

        # denominators: one ones-matmul per block over the accumulated pT
        accb = apool.tile([P, CW], BF16, tag="accb", name=f"accb{c}")
        nc.vector.tensor_copy(accb, acc)
        psd = psS.tile([P, CW], F32, tag="psS", name=f"psd{c}")
        rcp = rpool.tile([P, 2], F32, tag="rcp", name=f"rcp{c}")
        for j in range(2):
            nc.tensor.matmul(
                psd[:, j:j + 1], accb[:, j * P:(j + 1) * P], ones_b,
                start=True, stop=True)
            nc.vector.reciprocal(rcp[:, j:j + 1], psd[:, j:j + 1])
        pend[c] = (zt_sb, rcp)

    def do_vw(c):
        zt_sb, rcp = pend.pop(c)
        for j in range(2):
            osb = spool.tile([P, D], BF16, tag="osb", name=f"osb{c}_{j}")
            r0 = c * CW + j * P
            for h in range(2):
                po = psO.tile([P, D // 2], F32, tag="psO")
                for dt in range(DT):
                    nc.tensor.matmul(
                        po, zt_sb[:, dt, j * P:(j + 1) * P],
                        wv_sb[:, dt, h * (D // 2):(h + 1) * (D // 2)],
                        start=(dt == 0), stop=(dt == DT - 1))
                hs = slice(h * (D // 2), (h + 1) * (D // 2))
                # h-halves scaled out on different engines so the copies
                # overlap (scalar alone serialized the store tail)
                if h == 0:
                    nc.vector.tensor_scalar_mul(
                        osb[:, hs], po, rcp[:, j:j + 1])
                else:
                    nc.scalar.activation(
                        osb[:, hs], po,
                        mybir.ActivationFunctionType.Copy,
                        scale=rcp[:, j:j + 1])

            # full-row 2KB-per-partition stores on the sync hardware-DGE
            # ring (loads there are done by ~50us; keeping stores off the
            # scalar engine protects the exp/copy pipeline from blocking
            # DIRECT2D issues). The last chunk's stores are on the
            # critical tail: j=1 ships per-h above; j=0 in two row-groups
            # so two queues drain it in parallel.
            # the very last store goes via scalar (its copies ran there,
            # no exps remain to protect); everything else rides sync
            eng = nc.scalar if (c == NCH - 1 and j == 1) else nc.sync
            eng.dma_start(o_ap[r0:r0 + P, :], osb)

    # T1 slab 1 is deferred until after chunk 1 (its consumers are chunks
    # 2-3); VW trails by TWO chunks so wv's 2MB load leaves the HBM-
    # saturated front window
    do_t1(0)
    do_chunk(0)
    do_chunk(1)
    do_t1(1)
    do_chunk(2)
    do_vw(0)
    do_chunk(3)
    do_vw(1)
    do_vw(2)
    do_vw(3)


def build_program(cfg: Cfg):
    nc = bacc.Bacc("TRN2", dynamic_dma_scratch_size=2048)
    aps = {
        "xT": nc.dram_tensor("xT", [cfg.D, cfg.SEQ], BF16, kind="ExternalInput").ap(),
        "x": nc.dram_tensor("x", [cfg.SEQ, cfg.D], BF16, kind="ExternalInput").ap(),
        "xTq8": nc.dram_tensor(
            "xTq8", [cfg.NSL, P, cfg.DT * cfg.TS], F8,
            kind="ExternalInput").ap(),
        "wqk8": nc.dram_tensor(
            "wqk8", [P, cfg.DT * cfg.DT * P], F8, kind="ExternalInput").ap(),
        "wvT": nc.dram_tensor("wvT", [cfg.D, cfg.D], BF16, kind="ExternalInput").ap(),
        "mask": nc.dram_tensor(
            "mask", [P, cfg.n_mask_tiles * cfg.CW], F8,
            kind="ExternalInput").ap(),
        "o": nc.dram_tensor("o", [cfg.R, cfg.D], BF16, kind="ExternalOutput").ap(),
    }
    with tile.TileContext(nc) as tc:
        with ExitStack() as ctx:
            _emit(ctx, tc, cfg, aps)
    nc.compile()
    return nc


def make_mask(cfg: Cfg, qglob: np.ndarray) -> np.ndarray:
    """fp8 mask tiles for the last 4 k-tiles of each chunk: 1 = keep.
    Layout [P, n_mask_tiles*CW] (partition-major for one wide DMA)."""
    m = np.zeros((cfg.n_mask_tiles, P, cfg.CW), dtype=NP_F8)
    for c in range(cfg.NCH):
        qg = qglob[c * cfg.CW:(c + 1) * cfg.CW]  # [CW]
        E = cfg.ext(c)
        for i, t in enumerate(range(E - 4, E)):
            kg = np.arange(t * P, (t + 1) * P)  # [P]
            if i >= 2:
                # kernel computes only the second q-half for these tiles;
                # its mask sits in the slot's first 128 columns
                m[4 * c + i][:, 0:P] = (
                    kg[:, None] <= qg[None, P:]).astype(NP_F8)
            else:
                m[4 * c + i] = (kg[:, None] <= qg[None, :]).astype(NP_F8)
    return np.ascontiguousarray(m.transpose(1, 0, 2).reshape(P, -1))


def pack_dr_pairs(a: np.ndarray, w: int) -> np.ndarray:
    """[D, C] f32 -> fp8 [C//w, P, DT*w] with dt-pair interleave for
    DoubleRow: out[s, p, (dtp*2+i)*w + j] = a[(2*dtp+i)*P + p, s*w + j]."""
    Dd, C = a.shape
    DTP = Dd // P // 2
    return np.ascontiguousarray(
        a.reshape(DTP, 2, P, C // w, w)
        .transpose(3, 2, 0, 1, 4)
        .reshape(C // w, P, -1)).astype(NP_F8)


def make_core_inputs(cfg: Cfg, xT_bf, x_bf, x_f32, wqk8, wvT_bf, m: int):
    blocks = q_blocks(cfg, m)
    qglob = np.concatenate([np.arange(b * P, (b + 1) * P) for b in blocks])
    # xTq8[s, p, (dtp*2+i)*TS + j] = x[qglob[s*TS+j], (2*dtp+i)*P + p]
    xTq8 = pack_dr_pairs(np.ascontiguousarray(x_f32[qglob].T), cfg.TS)
    return {
        "xT": xT_bf,
        "x": x_bf,
        "xTq8": xTq8,
        "wqk8": wqk8,
        "wvT": wvT_bf,
        "mask": make_mask(cfg, qglob),
    }, qglob


_prog_cache = {}


def get_program(cfg: Cfg):
    if cfg not in _prog_cache:
        _prog_cache[cfg] = build_program(cfg)
    return _prog_cache[cfg]


def run(x, W_query, W_key, W_value, trace=False, trace_cores=None):
    """Returns (out [B, N, D], BassKernelResults)."""
    cfg = Cfg()
    B = x.shape[0]
    nc = get_program(cfg)
    x = np.asarray(x, dtype=np.float32)
    Wq = np.asarray(W_query, dtype=np.float32)
    Wk = np.asarray(W_key, dtype=np.float32)
    Wv = np.asarray(W_value, dtype=np.float32)
    # wqk8[p, ((o*DTP+dtp)*2+i)*P + col] = 32*wqk[(2dtp+i)*P+p, o*P+col]
    W32 = (WSCALE * (Wq.T @ Wk)).astype(np.float32)
    wqk8 = np.ascontiguousarray(
        W32.reshape(cfg.DT // 2, 2, P, cfg.DT, P)
        .transpose(2, 3, 0, 1, 4)
        .reshape(P, -1)).astype(NP_F8)
    wvT_bf = np.ascontiguousarray(Wv.T).astype(NP_BF16)

    in_maps = []
    qglobs = []
    for core in range(2 * B):
        b, m = core // 2, core % 2
        if m == 0:
            x_bf = x[b].astype(NP_BF16)
            xT_bf = np.ascontiguousarray(x[b].T).astype(NP_BF16)
        im, qglob = make_core_inputs(cfg, xT_bf, x_bf, x[b], wqk8, wvT_bf, m)
        in_maps.append(im)
        qglobs.append(qglob)

    res = run_bass_kernel_spmd(
        nc, in_maps, list(range(2 * B)), trace=trace,
        trace_cores=trace_cores)

    out = np.empty((B, cfg.SEQ, cfg.D), dtype=np.float32)
    for core in range(2 * B):
        b = core // 2
        out[b][qglobs[core]] = res.results[core]["o"].astype(np.float32)
    return out, res


def kernel(**inputs) -> np.ndarray:
    out, _ = run(
        inputs["x"], inputs["W_query"], inputs["W_key"], inputs["W_value"])
    return out
